# revision 56
# baseline (speedup 1.0000x reference)
"""TRN2 Bass kernel for nn_Decoder (6-layer 3D conv-transpose decoder), 8-core SPMD.

v5: L1 X-stationary GEMM + scatter-GEMM (cout-split), L2 cin-split with
kd-paired K=128 matmuls, single AllReduce of h2, L3..L6 voxel-split
per-core local frames, L6 via flat-shift R1 build + A/R9/C GEMMs.
Output: core c produces out planes [4c, 4c+3].

v3-v8 perf work: priority-staged weight DMAs (xx+w1 heads all queues,
L3+ weights load during the AR window), PE-warm filler matmuls to open
the HAM clock gate before L1, into the AR window, and briefly after it,
memsets hoisted onto idle engines during the load phase, L1 scatter
psum split in two groups with relus pipelined behind the scatter GEMMs
and the kd-pair dup copy split scalar/vector, per-half L4 psum tiles so
half-0 relus overlap half-1 matmuls, L5 emitted per output d-plane in
pairs so L6's R1 thirds overlap L5 compute, merged L5 relus via
interleaved-column APs, L6 Asb split 5+1 planes for earlier R9 gather,
per-pair output DMAs in L6's C-GEMM, Shared-addr-space AllReduce output
(faster HBM collective path), and a trimmed TC exit (no per-sem clear
storm; sems re-zero at NEFF load).
"""
import numpy as np

F16 = np.float16
N_CORES = 8
N_WARM0 = 14   # PE-warm matmuls bridging preamble -> L1
N_WARM1 = 16   # PE-warm matmuls into the AllReduce window
N_WARM2 = 14   # PE-warm matmuls re-opening the gate after the AllReduce

# ---------------- host prep ----------------
def _prep(inputs):
    x = np.asarray(inputs['x']).reshape(1024, 4, 4, 4)
    w = [np.asarray(inputs[f'w{i}']) for i in range(1, 7)]
    P = {}
    P['xx'] = np.ascontiguousarray(
        x.reshape(8, 128, 64).transpose(1, 0, 2)).astype(F16)  # [p, kb, vox]
    w1 = w[0].reshape(27, 1024, 512)
    # per-core cout slice, device layout [p(cin%128), kb, 27*64]
    P['w1r'] = [np.ascontiguousarray(
        w1[:, :, c*64:(c+1)*64].reshape(27, 8, 128, 64)
        .transpose(2, 1, 0, 3).reshape(128, 8, 27*64)).astype(F16)
        for c in range(N_CORES)]
    def s_axis(k):
        S = np.zeros((4, 4), np.float32)
        for o in range(4):
            m = o - 1 if k == 0 else o
            if 0 <= m < 4: S[m, o] = 1
        return S
    smat = np.zeros((64, 27, 64), np.float32)
    for t in range(27):
        kd, kh, kw = t//9, (t//3) % 3, t % 3
        smat[:, t, :] = np.kron(np.kron(s_axis(kd), s_axis(kh)), s_axis(kw))
    P['smat'] = smat.astype(F16)
    # L2: kd-paired packing. pairs (kd=0, kd=1) on 128 rows, kd=2 single on 64.
    w2 = w[1].reshape(27, 512, 256)
    P['w2p'], P['w2s'] = [], []
    for c in range(N_CORES):
        rows = slice(64*c, 64*c+64)
        w2p_h = np.zeros((128, 9, 2, 128), np.float32)
        w2s_h = np.zeros((64, 9, 2, 128), np.float32)
        for a in range(9):
            for mb in range(2):
                cols = slice(mb*128, (mb+1)*128)
                w2p_h[0:64, a, mb, :] = w2[a, rows, cols]
                w2p_h[64:128, a, mb, :] = w2[9+a, rows, cols]
                w2s_h[:, a, mb, :] = w2[18+a, rows, cols]
        P['w2p'].append(w2p_h.astype(F16))
        P['w2s'].append(w2s_h.astype(F16))
    P['w3r'] = np.ascontiguousarray(
        w[2].reshape(27, 2, 128, 128).transpose(2, 0, 1, 3)).astype(F16)  # [p, t, kb, c]
    P['w4r'] = np.ascontiguousarray(
        w[3].reshape(27, 128, 64).transpose(1, 0, 2)).astype(F16)  # [p, t, c]
    w5t = w[4].reshape(27, 64, 32)
    w5p_h = np.zeros((128, 9, 32), np.float32)
    w5s_h = np.zeros((64, 9, 32), np.float32)
    for a in range(9):
        w5p_h[0:64, a, :] = w5t[a]        # kd=0
        w5p_h[64:128, a, :] = w5t[18+a]   # kd=2
        w5s_h[:, a, :] = w5t[9+a]         # kd=1
    P['w5p'] = w5p_h.astype(F16)
    P['w5s'] = w5s_h.astype(F16)
    w6 = w[5].reshape(27, 32)
    w6a = np.zeros((96, 9), np.float32)
    for t in range(27):
        kd, kh, kw = t//9, (t//3) % 3, t % 3
        w6a[kw*32:(kw+1)*32, kh*3+kd] = w6[t]
    P['w6a'] = w6a.astype(F16)
    # per-core masks [128, 4] f32: cols = (h5p pd1lm0, h5p pd0lm2, h4p lj0, h3p pd0lm2)
    P['maskt'] = [np.broadcast_to(np.array(
        [1.0 if c > 0 else 0.0, 1.0 if c < 7 else 0.0,
         1.0 if c > 0 else 0.0, 1.0 if c < 7 else 0.0], np.float32), (128, 4)).copy()
        for c in range(N_CORES)]
    return P

# ---------------- device program ----------------
_CACHE = {}

def _build():
    import concourse.bass as bass
    import concourse.mybir as mybir
    import concourse.tile as tile
    from bass_rust import ScopedClock

    import bass_rust as _br

    class TC(tile.TileContext):
        # walrus's setupSyncWait rejects instructions carrying more than ~2
        # sem-waits; hoist excess waits onto single-wait NoOps inserted just
        # before the instruction on the same engine. Also: single-shot
        # kernel, so skip the exit-time sem clear storm (~5us of per-sem
        # EVENT_SEMAPHOREs) — sems are re-zeroed at NEFF load.
        def _drain_and_barrier(self, tick_clock, wait_clock):
            drain_inst = self.nc.sync.drain()
            wait_clock.add_sem_waits(
                drain_inst.ins, ScopedClock({None: tick_clock.global_clock}))
            self.nc.all_engine_barrier()
            popped = self.nc._tile_sem_poison_stack.pop()
            assert popped is self._sem_poison
            nc = self.nc
            for fn in nc.m.functions:
                for bb in fn.blocks:
                    il = bb.instructions
                    k = 0
                    while k < len(il):
                        inst = il[k]
                        si = inst.sync_info
                        if si is not None and si.on_wait and len(si.on_wait) > 1:
                            waits = list(si.on_wait)
                            for w in waits[:-1]:
                                nop = _br.InstNoOp(name=nc.get_next_instruction_name(),
                                                   ins=[], outs=[])
                                nop.engine = inst.engine
                                nop.sync_info = mybir.SyncInfo(on_wait=[w], on_update=[])
                                il.insert(k, nop)
                                k += 1
                            inst.sync_info = mybir.SyncInfo(on_wait=[waits[-1]],
                                                            on_update=list(si.on_update or []))
                        k += 1

    fp16 = mybir.dt.float16
    f32 = mybir.dt.float32
    RELU = mybir.ActivationFunctionType.Relu
    IDENT = mybir.ActivationFunctionType.Identity
    TAP2 = {0: [(0, -1), (2, 0)], 1: [(1, 0)]}

    nc = bass.Bass(num_devices=N_CORES)
    dp = nc.declare_dram_parameter
    xx_d = dp("xx", [128, 8, 64], fp16, isOutput=False)
    w1_d = dp("w1r", [128, 8, 1728], fp16, isOutput=False)
    sm_d = dp("smat", [64, 27, 64], fp16, isOutput=False)
    w2p_d = dp("w2p", [128, 9, 2, 128], fp16, isOutput=False)
    w2s_d = dp("w2s", [64, 9, 2, 128], fp16, isOutput=False)
    w3_d = dp("w3r", [128, 27, 2, 128], fp16, isOutput=False)
    w4_d = dp("w4r", [128, 27, 64], fp16, isOutput=False)
    w5p_d = dp("w5p", [128, 9, 32], fp16, isOutput=False)
    w5s_d = dp("w5s", [64, 9, 32], fp16, isOutput=False)
    w6_d = dp("w6a", [96, 9], fp16, isOutput=False)
    o9_d = dp("ones9", [9, 1], fp16, isOutput=False)
    mk_d = dp("maskt", [128, 4], f32, isOutput=False)
    y_d = dp("y", [1, 4096], f32, isOutput=True)

    with TC(nc) as tc:
        with (tc.tile_pool(name="w", bufs=1) as wp,
              tc.tile_pool(name="act", bufs=1) as ap,
              tc.tile_pool(name="dram", bufs=1, space="DRAM") as dr):
            # ---- loads staged by priority: L1 needs xx+w1 immediately; w2
            # next (~L2 start); sm mid-L1; w3..w6 have 70us+ of slack. Each
            # queue's head carries only phase-1 bytes so HBM bandwidth goes
            # to w1 first.
            w1 = wp.tile([128, 8, 1728], fp16)
            xx = wp.tile([128, 8, 64], fp16)
            sm = wp.tile([64, 27, 64], fp16)
            w2p = wp.tile([128, 9, 2, 128], fp16)
            w2s = wp.tile([64, 9, 2, 128], fp16)
            w3 = wp.tile([128, 27, 2, 128], fp16)
            w4 = wp.tile([128, 27, 64], fp16)
            w5p = wp.tile([128, 9, 32], fp16)
            w5s = wp.tile([64, 9, 32], fp16)
            w6 = wp.tile([96, 9], fp16)
            o9 = wp.tile([9, 1], fp16)
            mk = wp.tile([128, 4], f32)
            wt = wp.tile([128, 512], fp16)  # PE-warm garbage operand
            # phase 1: xx + w1 (8 kb-chunks round-robin on 3 queues, kb order)
            nc.sync.dma_start(out=xx[:], in_=xx_d[:])
            nc.sync.dma_start(out=w1[:, 0:1], in_=w1_d[:, 0:1])
            nc.scalar.dma_start(out=w1[:, 1:2], in_=w1_d[:, 1:2])
            nc.gpsimd.dma_start(out=w1[:, 2:3], in_=w1_d[:, 2:3])
            nc.sync.dma_start(out=w1[:, 3:4], in_=w1_d[:, 3:4])
            nc.scalar.dma_start(out=w1[:, 4:5], in_=w1_d[:, 4:5])
            nc.gpsimd.dma_start(out=w1[:, 5:6], in_=w1_d[:, 5:6])
            nc.sync.dma_start(out=w1[:, 6:7], in_=w1_d[:, 6:7])
            nc.scalar.dma_start(out=w1[:, 7:8], in_=w1_d[:, 7:8])
            # phase 2: sm (needed by L1 scatter), w2, mk
            nc.sync.dma_start(out=sm[:], in_=sm_d[:])
            nc.scalar.dma_start(out=w2p[:], in_=w2p_d[:])
            nc.gpsimd.dma_start(out=w2s[:], in_=w2s_d[:])
            nc.sync.dma_start(out=mk[:], in_=mk_d[:])
            # phase 3: late weights (L3+), queued behind the above
            nc.sync.dma_start(out=w3[:, :, 0], in_=w3_d[:, :, 0])
            nc.scalar.dma_start(out=w3[:, :, 1], in_=w3_d[:, :, 1])
            nc.gpsimd.dma_start(out=w4[:], in_=w4_d[:])
            nc.gpsimd.dma_start(out=w5p[:], in_=w5p_d[:])
            nc.gpsimd.dma_start(out=w5s[:], in_=w5s_d[:])
            nc.gpsimd.dma_start(out=w6[:], in_=w6_d[:])
            nc.gpsimd.dma_start(out=o9[:], in_=o9_d[:])

            # ---- tiles + memsets hoisted to the idle load window. Vector
            # covers the early-needed frames; gpsimd (idle 15-55us) covers
            # the L3+ frames.
            h2frame = ap.tile([128, 2, 14, 64], fp16)
            h2p = ap.tile([128, 2, 7, 9, 9], fp16)
            h1p = ap.tile([128, 10, 10, 10], fp16)
            h3p = ap.tile([128, 8, 18, 18], fp16)
            h4p = ap.tile([128, 6, 17, 17], fp16)
            h5p = ap.tile([32, 8, 34, 34], fp16)
            nc.vector.memset(wt[:], 0.125)
            nc.vector.memset(h1p[:], 0.0)
            nc.vector.memset(h2frame[:], 0.0)
            nc.vector.memset(h2p[:], 0.0)
            nc.gpsimd.memset(h3p[:], 0.0)
            nc.gpsimd.memset(h4p[:], 0.0)
            nc.gpsimd.memset(h5p[:, 0], 0.0)
            nc.gpsimd.memset(h5p[:, 7], 0.0)
            nc.gpsimd.memset(h5p[:, 1:7, 0, :], 0.0)
            nc.gpsimd.memset(h5p[:, 1:7, 33, :], 0.0)
            nc.gpsimd.memset(h5p[:, 1:7, 1:33, 0], 0.0)
            nc.gpsimd.memset(h5p[:, 1:7, 1:33, 33], 0.0)
            pid = nc.partition_id()
            # ---- PE warm-up: open the HAM clock gate before L1 arrives
            psw_cm = tc.tile_pool(name="psw", bufs=1, space="PSUM")
            psw = psw_cm.__enter__()
            wps = psw.tile([128, 512], f32)
            for _ in range(N_WARM0):
                nc.tensor.matmul(wps[:], wt[:, 0:128], wt[:],
                                 start=True, stop=True,
                                 skip_group_check=True)
            # ---- L1: T-GEMM + scatter-GEMM
            sc_l1 = nc.enter_named_scope("L1", False)
            with tc.tile_pool(name="ps1", bufs=1, space="PSUM") as ps1:
                Tps = ps1.tile([64, 4, 512], f32)
                Tsb = ap.tile([64, 1728], fp16)
                for kb in range(8):
                    for ch in range(4):
                        nc.tensor.matmul(Tps[:, ch, 0:432], xx[:, kb, :],
                                         w1[:, kb, ch*432:(ch+1)*432],
                                         start=(kb == 0), stop=(kb == 7),
                                         skip_group_check=True)
                # evacuate in halves split across scalar+vector
                nc.scalar.activation(Tsb[:, 0:864].rearrange("p (a b) -> p a b", a=2),
                                     Tps[:, 0:2, 0:432], IDENT)
                nc.vector.tensor_copy(Tsb[:, 864:1728].rearrange("p (a b) -> p a b", a=2),
                                      Tps[:, 2:4, 0:432])
                tap_class = [((0 if t//9 != 1 else 1)*4 + (0 if (t//3) % 3 != 1 else 1)*2
                             + (0 if t % 3 != 1 else 1)) for t in range(27)]
                cls_taps = {}
                for t in range(27): cls_taps.setdefault(tap_class[t], []).append(t)
                # two psum tiles (cls 0-3 / 4-7) so the first relus start
                # while the second tile's scatter matmuls still run
                h1ps_t = [ps1.tile([64, 4, 64], f32, tag=f"h1ps{g}",
                                   name=f"h1ps{g}")
                          for g in range(2)]
                for g in range(2):
                    for cls in range(4*g, 4*g+4):
                        taps = cls_taps[cls]
                        for j, t in enumerate(taps):
                            nc.tensor.matmul(h1ps_t[g][:, cls % 4, :],
                                             Tsb[:, t*64:(t+1)*64], sm[:, t, :],
                                             start=(j == 0),
                                             stop=(j == len(taps)-1),
                                             skip_group_check=True)
                    for cls in range(4*g, 4*g+4):
                        pd, ph, pw = cls//4, (cls//2) % 2, cls % 2
                        src = h1ps_t[g][:, cls % 4, :].rearrange(
                            "p (a b c) -> p a b c", a=4, b=4)
                        dlo = h1p[0:64, 1+pd:9:2, 1+ph:9:2, 1+pw:9:2]
                        if cls % 2 == 0:
                            nc.scalar.activation(dlo, src, RELU)
                        else:
                            nc.vector.tensor_relu(dlo, src)
                # upper half = lower shifted +1 in d (kd-pairing), split
                # across both engines
                nc.vector.tensor_copy(h1p[64:128, 0:4], h1p[0:64, 1:5])
                nc.scalar.activation(h1p[64:128, 4:9], h1p[0:64, 5:10], IDENT)
            nc.leave_named_scope("L1", sc_l1[0], False)
            # ---- L2 (cin-split, kd-paired K=128) + single AllReduce
            sc_l2 = nc.enter_named_scope("L2", False)
            h2in_d = dr.tile([128, 1024], fp16, name="h2in")
            h2out_d = dr.tile([128, 1024], fp16, name="h2out",
                              addr_space="Shared")
            with tc.tile_pool(name="ps2", bufs=1, space="PSUM") as ps2:
                h2ps = ps2.tile([128, 2, 8, 8, 8], f32)
                h2sb = ap.tile([128, 2, 512], fp16)
                for mb in range(2):
                    for a in range(9):
                        kh, kw = a // 3, a % 3
                        nc.tensor.matmul(h2ps[:, mb], w2p[:, a, mb, :],
                                         h1p[:, 0:8, kh:kh+8, kw:kw+8],
                                         start=(a == 0), stop=False,
                                         skip_group_check=True)
                    for a in range(9):
                        kh, kw = a // 3, a % 3
                        nc.tensor.matmul(h2ps[:, mb], w2s[:, a, mb, :],
                                         h1p[0:64, 2:10, kh:kh+8, kw:kw+8],
                                         start=False, stop=(a == 8),
                                         skip_group_check=True)
                    if mb == 0:
                        nc.scalar.activation(h2sb[:, mb, :],
                                             h2ps[:, mb].rearrange("p a b c -> p (a b c)"),
                                             IDENT)
                    else:
                        nc.vector.tensor_copy(h2sb[:, mb, :],
                                              h2ps[:, mb].rearrange("p a b c -> p (a b c)"))
                nc.sync.dma_start(out=h2in_d[:, 0:512], in_=h2sb[:, 0, :])
                nc.scalar.dma_start(out=h2in_d[:, 512:1024], in_=h2sb[:, 1, :])
                nc.gpsimd.collective_compute(
                    "AllReduce", mybir.AluOpType.add,
                    replica_groups=[list(range(N_CORES))],
                    ins=[h2in_d.opt()], outs=[h2out_d.opt()])
            nc.leave_named_scope("L2", sc_l2[0], False)
            # ---- keep the PE array hot into the AllReduce window; reading
            # h2sb pins these after L2 so the scheduler can't hoist them
            for _ in range(N_WARM1):
                nc.tensor.matmul(wps[:], wt[:, 0:128],
                                 h2sb.rearrange("p a b -> p (a b)")[:, 0:512],
                                 start=True, stop=True,
                                 skip_group_check=True)
            sc_ar = nc.enter_named_scope("AR", False)
            # unpack both halves in parallel: 2 DMAs on separate queues, relu
            # split across scalar and vector engines (dynamic 7-plane window)
            h2ov = h2out_d.rearrange("p (a d v) -> p a d v", a=2, d=8)
            nc.sync.dma_start(out=h2frame[:, 0, 3:11, :], in_=h2ov[:, 0])
            nc.scalar.dma_start(out=h2frame[:, 1, 3:11, :], in_=h2ov[:, 1])
            # small warm bridge: re-open the clock gate while the unpack
            # relus run, without materially delaying L3's first matmul
            for _ in range(N_WARM2):
                nc.tensor.matmul(wps[:], wt[:, 0:128],
                                 h2frame[:, 0, 3:11, :].rearrange(
                                     "p d v -> p (d v)"),
                                 start=True, stop=True,
                                 skip_group_check=True)
            nc.scalar.activation(
                h2p[:, 0, :, 1:9, 1:9],
                h2frame[:, 0, bass.ds(pid, 7), :].rearrange(
                    "p d (y z) -> p d y z", y=8),
                RELU)
            nc.vector.tensor_relu(
                h2p[:, 1, :, 1:9, 1:9],
                h2frame[:, 1, bass.ds(pid, 7), :].rearrange(
                    "p d (y z) -> p d y z", y=8))
            psw_cm.__exit__(None, None, None)
            nc.leave_named_scope("AR", sc_ar[0], False)
            sc_l3 = nc.enter_named_scope("L3", False)
            # ---- L3 (stride-2, vox-split)
            with tc.tile_pool(name="ps3", bufs=2, space="PSUM") as ps3:
                for cls in range(8):
                    pd, ph, pw = cls//4, (cls//2) % 2, cls % 2
                    units = [(kd*9+kh*3+kw, od, oh, ow, kb)
                             for kd, od in TAP2[pd] for kh, oh in TAP2[ph]
                             for kw, ow in TAP2[pw] for kb in range(2)]
                    h3ps = ps3.tile([128, 3, 8, 8], f32, tag="h3ps")
                    for j, (t, od, oh, ow, kb) in enumerate(units):
                        nc.tensor.matmul(h3ps[:], w3[:, t, kb, :],
                                         h2p[:, kb, od+2:od+5, oh+1:oh+9, ow+1:ow+9],
                                         start=(j == 0), stop=(j == len(units)-1),
                                         skip_group_check=True)
                    if pd == 1:
                        # no masks: all three lm planes in one vector relu
                        nc.vector.tensor_relu(h3p[:, 2:8:2, 1+ph:17:2, 1+pw:17:2],
                                              h3ps[:])
                    else:
                        nc.vector.tensor_relu(h3p[:, 1:5:2, 1+ph:17:2, 1+pw:17:2],
                                              h3ps[:, 0:2])
                        nc.scalar.activation(h3p[:, 5, 1+ph:17:2, 1+pw:17:2],
                                             h3ps[:, 2], RELU, scale=mk[0:128, 3:4])
            nc.leave_named_scope("L3", sc_l3[0], False)
            sc_l4 = nc.enter_named_scope("L4", False)
            # ---- L4 (stride-1); output duplicated to upper partitions with
            # +1 d-shift so L5 can pair taps (kd0, kd2) at K=128
            with tc.tile_pool(name="ps4", bufs=2, space="PSUM") as ps4:
                # separate psum tile per half so half 0's relus pipeline
                # with half 1's matmuls (tile-granular deps)
                for half in range(2):
                    h4ps = ps4.tile([64, 2, 16, 16], f32, tag="h4ps")
                    for t in range(27):
                        kd, kh, kw = t//9, (t//3) % 3, t % 3
                        nc.tensor.matmul(h4ps[:], w4[:, t, :],
                                         h3p[:, kd+1+2*half:kd+3+2*half, kh:kh+16, kw:kw+16],
                                         start=(t == 0), stop=(t == 26),
                                         skip_group_check=True)
                    # lower half: planes 1..4; upper half: same data shifted
                    # -1 plane (upper[d] = lower[d+1]) written from PSUM
                    lo, hi = 2*half + 1, 2*half + 3
                    if half == 0:
                        nc.scalar.activation(h4p[0:64, 1, 1:17, 1:17],
                                             h4ps[:, 0], RELU,
                                             scale=mk[0:64, 2:3])
                        nc.scalar.activation(h4p[64:128, 0, 1:17, 1:17],
                                             h4ps[:, 0], RELU,
                                             scale=mk[64:128, 2:3])
                        nc.vector.tensor_relu(h4p[0:64, 2, 1:17, 1:17],
                                              h4ps[:, 1])
                        nc.vector.tensor_relu(h4p[64:128, 1, 1:17, 1:17],
                                              h4ps[:, 1])
                    else:
                        nc.vector.tensor_relu(h4p[0:64, 3:5, 1:17, 1:17],
                                              h4ps[:])
                        nc.scalar.activation(h4p[64:128, 2:4, 1:17, 1:17],
                                             h4ps[:], RELU)
            nc.leave_named_scope("L4", sc_l4[0], False)
            sc_l5 = nc.enter_named_scope("L5", False)
            # ---- L5 (stride-2), emitted per output d-plane in pairs
            # (1,2),(3,4),(5,6) so each R1 third (L6 input) can start its
            # DMA while later planes still compute.
            R1p = ap.tile([96, 6944], fp16)
            h5flat = h5p[:, 1:7].rearrange("p a y x -> p (a y x)")
            dma_q = [nc.sync, nc.scalar, nc.gpsimd]
            with tc.tile_pool(name="ps5", bufs=2, space="PSUM") as ps5:
                for pair in range(3):
                    for dp in (1 + 2*pair, 2 + 2*pair):
                        pd = 0 if dp % 2 == 0 else 1
                        lm = (dp - 2)//2 if pd == 0 else (dp - 1)//2
                        w5w = w5p if pd == 0 else w5s
                        rows = slice(0, 128) if pd == 0 else slice(0, 64)
                        h5ps = ps5.tile([32, 4, 256], f32, tag="h5ps")
                        for ci in range(4):
                            ph, pw = ci // 2, ci % 2
                            hw_units = [(kh*3+kw, oh, ow)
                                        for kh, oh in TAP2[ph]
                                        for kw, ow in TAP2[pw]]
                            for j, (a, oh, ow) in enumerate(hw_units):
                                nc.tensor.matmul(
                                    h5ps[:, ci].rearrange("p (y z) -> p y z", y=16),
                                    w5w[rows, a, :],
                                    h4p[rows, 1+lm:2+lm, oh+1:oh+17, ow+1:ow+17],
                                    start=(j == 0),
                                    stop=(j == len(hw_units)-1),
                                    skip_group_check=True)
                        for ph in range(2):
                            # merge the two pw classes: psum [c,(y,x)] pairs
                            # interleave into the x'=2x+pw columns of h5p
                            dst = h5p[:, dp, 1+ph:33:2, 1:33].rearrange(
                                "p y (x c) -> p y x c", c=2)
                            src = h5ps[:, 2*ph:2*ph+2].rearrange(
                                "p c (y x) -> p y x c", y=16)
                            if dp == 1:
                                nc.scalar.activation(dst, src, RELU,
                                                     scale=mk[0:32, 0:1])
                            elif dp == 6:
                                nc.scalar.activation(dst, src, RELU,
                                                     scale=mk[0:32, 1:2])
                            elif ph == 0:
                                nc.vector.tensor_relu(dst, src)
                            else:
                                nc.scalar.activation(dst, src, RELU)
                    # R1 third for this plane pair (kw-shifted flat copies)
                    lo, hi = pair*2312, pair*2312 + 2312
                    for kw in range(3):
                        off = 2 - kw
                        dma_q[kw].dma_start(
                            out=R1p[kw*32:(kw+1)*32, off+lo:off+hi],
                            in_=h5flat[:, lo:hi])
            nc.leave_named_scope("L5", sc_l5[0], False)
            sc_l6 = nc.enter_named_scope("L6", False)
            # ---- L6: A-GEMM on strided view, R9 gather (2KB runs), C-GEMM.
            Av = R1p[:, 2:6938].rearrange("p (a y x) -> p a y x", a=6, y=34)
            # Asb split 5+1 planes: R9 rows with kd<2 only touch AsbA, so
            # their gather DMAs fire while plane 5's A-GEMM still runs
            AsbA = ap.tile([9, 5, 34, 32], fp16)
            AsbB = ap.tile([9, 1, 34, 32], fp16)
            ychunks = [(0, 12), (12, 23), (23, 34)]
            with tc.tile_pool(name="ps6", bufs=3, space="PSUM") as ps6:
                k = 0
                for a in range(6):
                    for (y0, y1) in ychunks:
                        n = (y1 - y0) * 32
                        Aps = ps6.tile([9, 384], f32, tag="aps")
                        nc.tensor.matmul(Aps[:, 0:n].rearrange("p (y x) -> p y x", x=32),
                                         w6[:], Av[:, a, y0:y1, 0:32],
                                         start=True, stop=True, skip_group_check=True)
                        dst = (AsbA[:, a, y0:y1, :] if a < 5
                               else AsbB[:, 0, y0:y1, :])
                        src = Aps[:, 0:n].rearrange("p (y x) -> p y x", x=32)
                        if k % 2 == 0:
                            nc.scalar.activation(dst, src, IDENT)
                        else:
                            nc.vector.tensor_copy(dst, src)
                        k += 1
                R9 = ap.tile([9, 4, 32, 32], fp16)
                qi = 0
                for a in range(9):
                    kh, kd = a // 3, a % 3
                    if kd < 2:
                        dma_q[qi % 3].dma_start(
                            out=R9[a:a+1],
                            in_=AsbA[a:a+1, kd:kd+4, kh:kh+32, :])
                        qi += 1
                for a in range(9):
                    kh, kd = a // 3, a % 3
                    if kd == 2:
                        dma_q[qi % 3].dma_start(
                            out=R9[a:a+1, 0:3],
                            in_=AsbA[a:a+1, 2:5, kh:kh+32, :])
                        qi += 1
                        dma_q[qi % 3].dma_start(
                            out=R9[a:a+1, 3:4],
                            in_=AsbB[a:a+1, 0:1, kh:kh+32, :])
                        qi += 1
                ysb = ap.tile([1, 4096], f32)
                # hold the clock gate open across the R9 gather gap so
                # the C-GEMM runs at full rate
                pw3_cm = tc.tile_pool(name="psw3", bufs=1, space="PSUM")
                pw3 = pw3_cm.__enter__()
                wps3 = pw3.tile([128, 512], f32, name="wps3")
                for _ in range(8):
                    nc.tensor.matmul(wps3[:], wt[:, 0:128], wt[:],
                                     start=True, stop=True,
                                     skip_group_check=True)
                R9f = R9.rearrange("p a b c -> p (a b c)")
                for c in range(8):
                    Cps = ps6.tile([1, 512], f32, tag="cps")
                    nc.tensor.matmul(Cps[:], o9[:], R9f[:, c*512:(c+1)*512],
                                     start=True, stop=True, skip_group_check=True)
                    if c % 2 == 0:
                        nc.scalar.activation(ysb[:, c*512:(c+1)*512], Cps[:], IDENT)
                    else:
                        nc.vector.tensor_copy(ysb[:, c*512:(c+1)*512], Cps[:])
                    if c % 2 == 1:
                        # ship each completed 1KB pair immediately
                        dma_q[(c // 2) % 3].dma_start(
                            out=y_d[:, (c-1)*512:(c+1)*512],
                            in_=ysb[:, (c-1)*512:(c+1)*512])
                pw3_cm.__exit__(None, None, None)
    nc.leave_named_scope("L6", sc_l6[0], False)
    return nc

def _get_nc():
    if 'nc' not in _CACHE:
        _CACHE['nc'] = _build()
    return _CACHE['nc']

def run(inputs, trace=False):
    from concourse.bass_utils import run_bass_kernel_spmd
    P = _prep(inputs)
    in_maps = []
    for c in range(N_CORES):
        in_maps.append({
            "xx": P['xx'], "w1r": P['w1r'][c], "smat": P['smat'],
            "w2p": P['w2p'][c], "w2s": P['w2s'][c], "w3r": P['w3r'],
            "w4r": P['w4r'], "w5p": P['w5p'], "w5s": P['w5s'], "w6a": P['w6a'],
            "ones9": np.ones((9, 1), np.float16), "maskt": P['maskt'][c],
        })
    res = run_bass_kernel_spmd(_get_nc(), in_maps, list(range(N_CORES)), trace=trace)
    out = np.concatenate([res.results[c]["y"].reshape(4, 32, 32) for c in range(N_CORES)], axis=0)
    return out[None, None].astype(np.float32), res

def kernel(**inputs):
    out, _ = run(inputs, trace=False)
    return out



# revision 59
# speedup vs baseline: 1.1740x; 1.1740x over previous
"""TRN2 Bass kernel for nn_Decoder (6-layer 3D conv-transpose decoder), 8-core SPMD.

v5: L1 X-stationary GEMM + scatter-GEMM (cout-split), L2 cin-split with
kd-paired K=128 matmuls, single AllReduce of h2, L3..L6 voxel-split
per-core local frames, L6 via flat-shift R1 build + A/R9/C GEMMs.
Output: core c produces out planes [4c, 4c+3].

v3-v8 perf work: priority-staged weight DMAs (xx+w1 heads all queues,
L3+ weights load during the AR window), PE-warm filler matmuls to open
the HAM clock gate before L1, into the AR window, and briefly after it,
memsets hoisted onto idle engines during the load phase, L1 scatter
psum split in two groups with relus pipelined behind the scatter GEMMs
and the kd-pair dup copy split scalar/vector, per-half L4 psum tiles so
half-0 relus overlap half-1 matmuls, L5 emitted per output d-plane in
pairs so L6's R1 thirds overlap L5 compute, merged L5 relus via
interleaved-column APs, L6 Asb split 5+1 planes for earlier R9 gather,
per-pair output DMAs in L6's C-GEMM, Shared-addr-space AllReduce output
(faster HBM collective path), and a trimmed TC exit (no per-sem clear
storm; sems re-zero at NEFF load).
"""
import numpy as np

F16 = np.float16
N_CORES = 8
N_WARM0 = 14   # PE-warm matmuls bridging preamble -> L1
N_WARM1 = 16   # PE-warm matmuls into the AllReduce window
N_WARM2 = 14   # PE-warm matmuls re-opening the gate after the AllReduce

# ---------------- host prep ----------------
def _prep(inputs):
    x = np.asarray(inputs['x']).reshape(1024, 4, 4, 4)
    w = [np.asarray(inputs[f'w{i}']) for i in range(1, 7)]
    P = {}
    P['xx'] = np.ascontiguousarray(
        x.reshape(8, 128, 64).transpose(1, 0, 2)).astype(F16)  # [p, kb, vox]
    w1 = w[0].reshape(27, 1024, 512)
    # per-core cout slice, device layout [p(cin%128), kb, 27*64]
    P['w1r'] = [np.ascontiguousarray(
        w1[:, :, c*64:(c+1)*64].reshape(27, 8, 128, 64)
        .transpose(2, 1, 0, 3).reshape(128, 8, 27*64)).astype(F16)
        for c in range(N_CORES)]
    def s_axis(k):
        S = np.zeros((4, 4), np.float32)
        for o in range(4):
            m = o - 1 if k == 0 else o
            if 0 <= m < 4: S[m, o] = 1
        return S
    smat = np.zeros((64, 27, 64), np.float32)
    for t in range(27):
        kd, kh, kw = t//9, (t//3) % 3, t % 3
        smat[:, t, :] = np.kron(np.kron(s_axis(kd), s_axis(kh)), s_axis(kw))
    P['smat'] = smat.astype(F16)
    # L2: kd-paired packing. pairs (kd=0, kd=1) on 128 rows, kd=2 single on 64.
    w2 = w[1].reshape(27, 512, 256)
    P['w2p'], P['w2s'] = [], []
    for c in range(N_CORES):
        rows = slice(64*c, 64*c+64)
        w2p_h = np.zeros((128, 9, 2, 128), np.float32)
        w2s_h = np.zeros((64, 9, 2, 128), np.float32)
        for a in range(9):
            for mb in range(2):
                cols = slice(mb*128, (mb+1)*128)
                w2p_h[0:64, a, mb, :] = w2[a, rows, cols]
                w2p_h[64:128, a, mb, :] = w2[9+a, rows, cols]
                w2s_h[:, a, mb, :] = w2[18+a, rows, cols]
        P['w2p'].append(w2p_h.astype(F16))
        P['w2s'].append(w2s_h.astype(F16))
    P['w3r'] = np.ascontiguousarray(
        w[2].reshape(27, 2, 128, 128).transpose(2, 0, 1, 3)).astype(F16)  # [p, t, kb, c]
    P['w4r'] = np.ascontiguousarray(
        w[3].reshape(27, 128, 64).transpose(1, 0, 2)).astype(F16)  # [p, t, c]
    w5t = w[4].reshape(27, 64, 32)
    w5p_h = np.zeros((128, 9, 32), np.float32)
    w5s_h = np.zeros((64, 9, 32), np.float32)
    for a in range(9):
        w5p_h[0:64, a, :] = w5t[a]        # kd=0
        w5p_h[64:128, a, :] = w5t[18+a]   # kd=2
        w5s_h[:, a, :] = w5t[9+a]         # kd=1
    P['w5p'] = w5p_h.astype(F16)
    P['w5s'] = w5s_h.astype(F16)
    w6 = w[5].reshape(27, 32)
    w6a = np.zeros((96, 9), np.float32)
    for t in range(27):
        kd, kh, kw = t//9, (t//3) % 3, t % 3
        w6a[kw*32:(kw+1)*32, kh*3+kd] = w6[t]
    P['w6a'] = w6a.astype(F16)
    # per-core masks [128, 4] f32: cols = (h5p pd1lm0, h5p pd0lm2, h4p lj0, h3p pd0lm2)
    P['maskt'] = [np.broadcast_to(np.array(
        [1.0 if c > 0 else 0.0, 1.0 if c < 7 else 0.0,
         1.0 if c > 0 else 0.0, 1.0 if c < 7 else 0.0], np.float32), (128, 4)).copy()
        for c in range(N_CORES)]
    return P

# ---------------- device program ----------------
_CACHE = {}

def _build():
    import concourse.bass as bass
    import concourse.mybir as mybir
    import concourse.tile as tile
    from bass_rust import ScopedClock

    import bass_rust as _br

    class TC(tile.TileContext):
        # walrus's setupSyncWait rejects instructions carrying more than ~2
        # sem-waits; hoist excess waits onto single-wait NoOps inserted just
        # before the instruction on the same engine. Also: single-shot
        # kernel, so skip the exit-time sem clear storm (~5us of per-sem
        # EVENT_SEMAPHOREs) — sems are re-zeroed at NEFF load.
        def _drain_and_barrier(self, tick_clock, wait_clock):
            drain_inst = self.nc.sync.drain()
            wait_clock.add_sem_waits(
                drain_inst.ins, ScopedClock({None: tick_clock.global_clock}))
            self.nc.all_engine_barrier()
            popped = self.nc._tile_sem_poison_stack.pop()
            assert popped is self._sem_poison
            nc = self.nc
            for fn in nc.m.functions:
                for bb in fn.blocks:
                    il = bb.instructions
                    k = 0
                    while k < len(il):
                        inst = il[k]
                        si = inst.sync_info
                        if si is not None and si.on_wait and len(si.on_wait) > 1:
                            waits = list(si.on_wait)
                            for w in waits[:-1]:
                                nop = _br.InstNoOp(name=nc.get_next_instruction_name(),
                                                   ins=[], outs=[])
                                nop.engine = inst.engine
                                nop.sync_info = mybir.SyncInfo(on_wait=[w], on_update=[])
                                il.insert(k, nop)
                                k += 1
                            inst.sync_info = mybir.SyncInfo(on_wait=[waits[-1]],
                                                            on_update=list(si.on_update or []))
                        k += 1

    fp16 = mybir.dt.float16
    f32 = mybir.dt.float32
    RELU = mybir.ActivationFunctionType.Relu
    IDENT = mybir.ActivationFunctionType.Identity
    TAP2 = {0: [(0, -1), (2, 0)], 1: [(1, 0)]}

    nc = bass.Bass(num_devices=N_CORES)
    dp = nc.declare_dram_parameter
    xx_d = dp("xx", [128, 8, 64], fp16, isOutput=False)
    w1_d = dp("w1r", [128, 8, 1728], fp16, isOutput=False)
    sm_d = dp("smat", [64, 27, 64], fp16, isOutput=False)
    w2p_d = dp("w2p", [128, 9, 2, 128], fp16, isOutput=False)
    w2s_d = dp("w2s", [64, 9, 2, 128], fp16, isOutput=False)
    w3_d = dp("w3r", [128, 27, 2, 128], fp16, isOutput=False)
    w4_d = dp("w4r", [128, 27, 64], fp16, isOutput=False)
    w5p_d = dp("w5p", [128, 9, 32], fp16, isOutput=False)
    w5s_d = dp("w5s", [64, 9, 32], fp16, isOutput=False)
    w6_d = dp("w6a", [96, 9], fp16, isOutput=False)
    o9_d = dp("ones9", [9, 1], fp16, isOutput=False)
    mk_d = dp("maskt", [128, 4], f32, isOutput=False)
    y_d = dp("y", [1, 4096], f32, isOutput=True)

    with TC(nc) as tc:
        with (tc.tile_pool(name="w", bufs=1) as wp,
              tc.tile_pool(name="act", bufs=1) as ap,
              tc.tile_pool(name="dram", bufs=1, space="DRAM") as dr):
            # ---- loads staged by priority: L1 needs xx+w1 immediately; w2
            # next (~L2 start); sm mid-L1; w3..w6 have 70us+ of slack. Each
            # queue's head carries only phase-1 bytes so HBM bandwidth goes
            # to w1 first.
            w1 = wp.tile([128, 8, 1728], fp16)
            xx = wp.tile([128, 8, 64], fp16)
            sm = wp.tile([64, 27, 64], fp16)
            w2p = wp.tile([128, 9, 2, 128], fp16)
            w2s = wp.tile([64, 9, 2, 128], fp16)
            w3 = wp.tile([128, 27, 2, 128], fp16)
            w4 = wp.tile([128, 27, 64], fp16)
            w5p = wp.tile([128, 9, 32], fp16)
            w5s = wp.tile([64, 9, 32], fp16)
            w6 = wp.tile([96, 9], fp16)
            o9 = wp.tile([9, 1], fp16)
            mk = wp.tile([128, 4], f32)
            wt = wp.tile([128, 512], fp16)  # PE-warm garbage operand
            # phase 1: xx + w1 (8 kb-chunks round-robin on 3 queues, kb order)
            nc.sync.dma_start(out=xx[:], in_=xx_d[:])
            nc.sync.dma_start(out=w1[:, 0:1], in_=w1_d[:, 0:1])
            nc.scalar.dma_start(out=w1[:, 1:2], in_=w1_d[:, 1:2])
            nc.gpsimd.dma_start(out=w1[:, 2:3], in_=w1_d[:, 2:3])
            nc.sync.dma_start(out=w1[:, 3:4], in_=w1_d[:, 3:4])
            nc.scalar.dma_start(out=w1[:, 4:5], in_=w1_d[:, 4:5])
            nc.gpsimd.dma_start(out=w1[:, 5:6], in_=w1_d[:, 5:6])
            nc.sync.dma_start(out=w1[:, 6:7], in_=w1_d[:, 6:7])
            nc.scalar.dma_start(out=w1[:, 7:8], in_=w1_d[:, 7:8])
            # phase 2: sm (needed by L1 scatter), w2, mk
            nc.sync.dma_start(out=sm[:], in_=sm_d[:])
            nc.scalar.dma_start(out=w2p[:], in_=w2p_d[:])
            nc.gpsimd.dma_start(out=w2s[:], in_=w2s_d[:])
            nc.sync.dma_start(out=mk[:], in_=mk_d[:])
            # phase 3: late weights (L3+), queued behind the above
            nc.sync.dma_start(out=w3[:, :, 0], in_=w3_d[:, :, 0])
            nc.scalar.dma_start(out=w3[:, :, 1], in_=w3_d[:, :, 1])
            nc.gpsimd.dma_start(out=w4[:], in_=w4_d[:])
            nc.gpsimd.dma_start(out=w5p[:], in_=w5p_d[:])
            nc.gpsimd.dma_start(out=w5s[:], in_=w5s_d[:])
            nc.gpsimd.dma_start(out=w6[:], in_=w6_d[:])
            nc.gpsimd.dma_start(out=o9[:], in_=o9_d[:])

            # ---- tiles + memsets hoisted to the idle load window. Vector
            # covers the early-needed frames; gpsimd (idle 15-55us) covers
            # the L3+ frames.
            h2frame = ap.tile([128, 2, 14, 64], fp16)
            h2p = ap.tile([128, 2, 7, 9, 9], fp16)
            h1p = ap.tile([128, 10, 10, 10], fp16)
            h3p = ap.tile([128, 8, 18, 18], fp16)
            h4p = ap.tile([128, 6, 17, 17], fp16)
            h5p = ap.tile([32, 8, 34, 34], fp16)
            nc.vector.memset(wt[:], 0.125)
            nc.vector.memset(h1p[:], 0.0)
            nc.vector.memset(h2frame[:], 0.0)
            nc.vector.memset(h2p[:], 0.0)
            nc.gpsimd.memset(h3p[:], 0.0)
            nc.gpsimd.memset(h4p[:], 0.0)
            nc.gpsimd.memset(h5p[:, 0], 0.0)
            nc.gpsimd.memset(h5p[:, 7], 0.0)
            nc.gpsimd.memset(h5p[:, 1:7, 0, :], 0.0)
            nc.gpsimd.memset(h5p[:, 1:7, 33, :], 0.0)
            nc.gpsimd.memset(h5p[:, 1:7, 1:33, 0], 0.0)
            nc.gpsimd.memset(h5p[:, 1:7, 1:33, 33], 0.0)
            pid = nc.partition_id()
            # ---- PE warm-up: open the HAM clock gate before L1 arrives
            psw_cm = tc.tile_pool(name="psw", bufs=1, space="PSUM")
            psw = psw_cm.__enter__()
            wps = psw.tile([128, 512], f32)
            for _ in range(N_WARM0):
                nc.tensor.matmul(wps[:], wt[:, 0:128], wt[:],
                                 start=True, stop=True,
                                 skip_group_check=True)
            # ---- L1: T-GEMM + scatter-GEMM
            sc_l1 = nc.enter_named_scope("L1", False)
            with tc.tile_pool(name="ps1", bufs=1, space="PSUM") as ps1:
                Tps = ps1.tile([64, 4, 512], f32)
                Tsb = ap.tile([64, 1728], fp16)
                for kb in range(8):
                    for ch in range(4):
                        nc.tensor.matmul(Tps[:, ch, 0:432], xx[:, kb, :],
                                         w1[:, kb, ch*432:(ch+1)*432],
                                         start=(kb == 0), stop=(kb == 7),
                                         skip_group_check=True)
                # evacuate in halves split across scalar+vector
                nc.scalar.activation(Tsb[:, 0:864].rearrange("p (a b) -> p a b", a=2),
                                     Tps[:, 0:2, 0:432], IDENT)
                nc.vector.tensor_copy(Tsb[:, 864:1728].rearrange("p (a b) -> p a b", a=2),
                                      Tps[:, 2:4, 0:432])
                tap_class = [((0 if t//9 != 1 else 1)*4 + (0 if (t//3) % 3 != 1 else 1)*2
                             + (0 if t % 3 != 1 else 1)) for t in range(27)]
                cls_taps = {}
                for t in range(27): cls_taps.setdefault(tap_class[t], []).append(t)
                # two psum tiles (cls 0-3 / 4-7) so the first relus start
                # while the second tile's scatter matmuls still run
                h1ps_t = [ps1.tile([64, 4, 64], f32, tag=f"h1ps{g}",
                                   name=f"h1ps{g}")
                          for g in range(2)]
                for g in range(2):
                    for cls in range(4*g, 4*g+4):
                        taps = cls_taps[cls]
                        for j, t in enumerate(taps):
                            nc.tensor.matmul(h1ps_t[g][:, cls % 4, :],
                                             Tsb[:, t*64:(t+1)*64], sm[:, t, :],
                                             start=(j == 0),
                                             stop=(j == len(taps)-1),
                                             skip_group_check=True)
                    for cls in range(4*g, 4*g+4):
                        pd, ph, pw = cls//4, (cls//2) % 2, cls % 2
                        src = h1ps_t[g][:, cls % 4, :].rearrange(
                            "p (a b c) -> p a b c", a=4, b=4)
                        dlo = h1p[0:64, 1+pd:9:2, 1+ph:9:2, 1+pw:9:2]
                        if cls % 2 == 0:
                            nc.scalar.activation(dlo, src, RELU)
                        else:
                            nc.vector.tensor_relu(dlo, src)
                # upper half = lower shifted +1 in d (kd-pairing), split
                # across both engines
                nc.vector.tensor_copy(h1p[64:128, 0:4], h1p[0:64, 1:5])
                nc.scalar.activation(h1p[64:128, 4:9], h1p[0:64, 5:10], IDENT)
            nc.leave_named_scope("L1", sc_l1[0], False)
            # ---- L2 (cin-split, kd-paired K=128) + single AllReduce
            sc_l2 = nc.enter_named_scope("L2", False)
            h2in_d = dr.tile([128, 1024], fp16, name="h2in")
            h2out_d = dr.tile([128, 1024], fp16, name="h2out",
                              addr_space="Shared")
            with tc.tile_pool(name="ps2", bufs=1, space="PSUM") as ps2:
                h2ps = ps2.tile([128, 2, 8, 8, 8], f32)
                h2sb = ap.tile([128, 2, 512], fp16)
                for mb in range(2):
                    for a in range(9):
                        kh, kw = a // 3, a % 3
                        nc.tensor.matmul(h2ps[:, mb], w2p[:, a, mb, :],
                                         h1p[:, 0:8, kh:kh+8, kw:kw+8],
                                         start=(a == 0), stop=False,
                                         skip_group_check=True)
                    for a in range(9):
                        kh, kw = a // 3, a % 3
                        nc.tensor.matmul(h2ps[:, mb], w2s[:, a, mb, :],
                                         h1p[0:64, 2:10, kh:kh+8, kw:kw+8],
                                         start=False, stop=(a == 8),
                                         skip_group_check=True)
                    if mb == 0:
                        nc.scalar.activation(h2sb[:, mb, :],
                                             h2ps[:, mb].rearrange("p a b c -> p (a b c)"),
                                             IDENT)
                    else:
                        nc.vector.tensor_copy(h2sb[:, mb, :],
                                              h2ps[:, mb].rearrange("p a b c -> p (a b c)"))
                nc.sync.dma_start(out=h2in_d[:, 0:512], in_=h2sb[:, 0, :])
                nc.scalar.dma_start(out=h2in_d[:, 512:1024], in_=h2sb[:, 1, :])
                nc.gpsimd.collective_compute(
                    "AllReduce", mybir.AluOpType.add,
                    replica_groups=[list(range(N_CORES))],
                    ins=[h2in_d.opt()], outs=[h2out_d.opt()])
            nc.leave_named_scope("L2", sc_l2[0], False)
            # ---- keep the PE array hot into the AllReduce window; reading
            # h2sb pins these after L2 so the scheduler can't hoist them
            for _ in range(N_WARM1):
                nc.tensor.matmul(wps[:], wt[:, 0:128],
                                 h2sb.rearrange("p a b -> p (a b)")[:, 0:512],
                                 start=True, stop=True,
                                 skip_group_check=True)
            sc_ar = nc.enter_named_scope("AR", False)
            # unpack both halves in parallel: 2 DMAs on separate queues, relu
            # split across scalar and vector engines (dynamic 7-plane window)
            h2ov = h2out_d.rearrange("p (a d v) -> p a d v", a=2, d=8)
            nc.sync.dma_start(out=h2frame[:, 0, 3:11, :], in_=h2ov[:, 0])
            nc.scalar.dma_start(out=h2frame[:, 1, 3:11, :], in_=h2ov[:, 1])
            # small warm bridge: re-open the clock gate while the unpack
            # relus run, without materially delaying L3's first matmul
            for _ in range(N_WARM2):
                nc.tensor.matmul(wps[:], wt[:, 0:128],
                                 h2frame[:, 0, 3:11, :].rearrange(
                                     "p d v -> p (d v)"),
                                 start=True, stop=True,
                                 skip_group_check=True)
            nc.scalar.activation(
                h2p[:, 0, :, 1:9, 1:9],
                h2frame[:, 0, bass.ds(pid, 7), :].rearrange(
                    "p d (y z) -> p d y z", y=8),
                RELU)
            nc.vector.tensor_relu(
                h2p[:, 1, :, 1:9, 1:9],
                h2frame[:, 1, bass.ds(pid, 7), :].rearrange(
                    "p d (y z) -> p d y z", y=8))
            psw_cm.__exit__(None, None, None)
            nc.leave_named_scope("AR", sc_ar[0], False)
            sc_l3 = nc.enter_named_scope("L3", False)
            # ---- L3 (stride-2, vox-split)
            with tc.tile_pool(name="ps3", bufs=2, space="PSUM") as ps3:
                for cls in range(8):
                    pd, ph, pw = cls//4, (cls//2) % 2, cls % 2
                    units = [(kd*9+kh*3+kw, od, oh, ow, kb)
                             for kd, od in TAP2[pd] for kh, oh in TAP2[ph]
                             for kw, ow in TAP2[pw] for kb in range(2)]
                    h3ps = ps3.tile([128, 3, 8, 8], f32, tag="h3ps")
                    for j, (t, od, oh, ow, kb) in enumerate(units):
                        nc.tensor.matmul(h3ps[:], w3[:, t, kb, :],
                                         h2p[:, kb, od+2:od+5, oh+1:oh+9, ow+1:ow+9],
                                         start=(j == 0), stop=(j == len(units)-1),
                                         skip_group_check=True)
                    if pd == 1:
                        # no masks: all three lm planes in one vector relu
                        nc.vector.tensor_relu(h3p[:, 2:8:2, 1+ph:17:2, 1+pw:17:2],
                                              h3ps[:])
                    else:
                        nc.vector.tensor_relu(h3p[:, 1:5:2, 1+ph:17:2, 1+pw:17:2],
                                              h3ps[:, 0:2])
                        nc.scalar.activation(h3p[:, 5, 1+ph:17:2, 1+pw:17:2],
                                             h3ps[:, 2], RELU, scale=mk[0:128, 3:4])
            nc.leave_named_scope("L3", sc_l3[0], False)
            sc_l4 = nc.enter_named_scope("L4", False)
            # ---- L4 (stride-1); output duplicated to upper partitions with
            # +1 d-shift so L5 can pair taps (kd0, kd2) at K=128
            with tc.tile_pool(name="ps4", bufs=2, space="PSUM") as ps4:
                # separate psum tile per half so half 0's relus pipeline
                # with half 1's matmuls (tile-granular deps)
                for half in range(2):
                    h4ps = ps4.tile([64, 2, 16, 16], f32, tag="h4ps")
                    for t in range(27):
                        kd, kh, kw = t//9, (t//3) % 3, t % 3
                        nc.tensor.matmul(h4ps[:], w4[:, t, :],
                                         h3p[:, kd+1+2*half:kd+3+2*half, kh:kh+16, kw:kw+16],
                                         start=(t == 0), stop=(t == 26),
                                         skip_group_check=True)
                    # lower half: planes 1..4; upper half: same data shifted
                    # -1 plane (upper[d] = lower[d+1]) written from PSUM
                    lo, hi = 2*half + 1, 2*half + 3
                    if half == 0:
                        nc.scalar.activation(h4p[0:64, 1, 1:17, 1:17],
                                             h4ps[:, 0], RELU,
                                             scale=mk[0:64, 2:3])
                        nc.scalar.activation(h4p[64:128, 0, 1:17, 1:17],
                                             h4ps[:, 0], RELU,
                                             scale=mk[64:128, 2:3])
                        nc.vector.tensor_relu(h4p[0:64, 2, 1:17, 1:17],
                                              h4ps[:, 1])
                        nc.vector.tensor_relu(h4p[64:128, 1, 1:17, 1:17],
                                              h4ps[:, 1])
                    else:
                        nc.vector.tensor_relu(h4p[0:64, 3:5, 1:17, 1:17],
                                              h4ps[:])
                        nc.scalar.activation(h4p[64:128, 2:4, 1:17, 1:17],
                                             h4ps[:], RELU)
            nc.leave_named_scope("L4", sc_l4[0], False)
            sc_l5 = nc.enter_named_scope("L5", False)
            # ---- L5 (stride-2), emitted per output d-plane in pairs
            # (1,2),(3,4),(5,6) so each R1 third (L6 input) can start its
            # DMA while later planes still compute.
            R1p = ap.tile([96, 6944], fp16)
            h5flat = h5p[:, 1:7].rearrange("p a y x -> p (a y x)")
            dma_q = [nc.sync, nc.scalar, nc.gpsimd]
            with tc.tile_pool(name="ps5", bufs=2, space="PSUM") as ps5:
                for pair in range(3):
                    for dp in (1 + 2*pair, 2 + 2*pair):
                        pd = 0 if dp % 2 == 0 else 1
                        lm = (dp - 2)//2 if pd == 0 else (dp - 1)//2
                        w5w = w5p if pd == 0 else w5s
                        rows = slice(0, 128) if pd == 0 else slice(0, 64)
                        h5ps = ps5.tile([32, 4, 256], f32, tag="h5ps")
                        for ci in range(4):
                            ph, pw = ci // 2, ci % 2
                            hw_units = [(kh*3+kw, oh, ow)
                                        for kh, oh in TAP2[ph]
                                        for kw, ow in TAP2[pw]]
                            for j, (a, oh, ow) in enumerate(hw_units):
                                nc.tensor.matmul(
                                    h5ps[:, ci].rearrange("p (y z) -> p y z", y=16),
                                    w5w[rows, a, :],
                                    h4p[rows, 1+lm:2+lm, oh+1:oh+17, ow+1:ow+17],
                                    start=(j == 0),
                                    stop=(j == len(hw_units)-1),
                                    skip_group_check=True)
                        for ph in range(2):
                            # merge the two pw classes: psum [c,(y,x)] pairs
                            # interleave into the x'=2x+pw columns of h5p
                            dst = h5p[:, dp, 1+ph:33:2, 1:33].rearrange(
                                "p y (x c) -> p y x c", c=2)
                            src = h5ps[:, 2*ph:2*ph+2].rearrange(
                                "p c (y x) -> p y x c", y=16)
                            if dp == 1:
                                nc.scalar.activation(dst, src, RELU,
                                                     scale=mk[0:32, 0:1])
                            elif dp == 6:
                                nc.scalar.activation(dst, src, RELU,
                                                     scale=mk[0:32, 1:2])
                            elif ph == 0:
                                nc.vector.tensor_relu(dst, src)
                            else:
                                nc.scalar.activation(dst, src, RELU)
                    # R1 third for this plane pair (kw-shifted flat copies)
                    lo, hi = pair*2312, pair*2312 + 2312
                    for kw in range(3):
                        off = 2 - kw
                        dma_q[kw].dma_start(
                            out=R1p[kw*32:(kw+1)*32, off+lo:off+hi],
                            in_=h5flat[:, lo:hi])
            nc.leave_named_scope("L5", sc_l5[0], False)
            sc_l6 = nc.enter_named_scope("L6", False)
            # ---- L6: A-GEMM on strided view, R9 gather (2KB runs), C-GEMM.
            Av = R1p[:, 2:6938].rearrange("p (a y x) -> p a y x", a=6, y=34)
            # Asb split 5+1 planes: R9 rows with kd<2 only touch AsbA, so
            # their gather DMAs fire while plane 5's A-GEMM still runs
            AsbA = ap.tile([9, 5, 34, 32], fp16)
            AsbB = ap.tile([9, 1, 34, 32], fp16)
            ychunks = [(0, 12), (12, 23), (23, 34)]
            with tc.tile_pool(name="ps6", bufs=4, space="PSUM") as ps6:
                k = 0
                for a in range(6):
                    for (y0, y1) in ychunks:
                        n = (y1 - y0) * 32
                        Aps = ps6.tile([9, 384], f32, tag="aps")
                        nc.tensor.matmul(Aps[:, 0:n].rearrange("p (y x) -> p y x", x=32),
                                         w6[:], Av[:, a, y0:y1, 0:32],
                                         start=True, stop=True, skip_group_check=True)
                        dst = (AsbA[:, a, y0:y1, :] if a < 5
                               else AsbB[:, 0, y0:y1, :])
                        src = Aps[:, 0:n].rearrange("p (y x) -> p y x", x=32)
                        if k % 2 == 0:
                            nc.scalar.activation(dst, src, IDENT)
                        else:
                            nc.vector.tensor_copy(dst, src)
                        k += 1
                R9 = ap.tile([9, 4, 32, 32], fp16)
                qi = 0
                for a in range(9):
                    kh, kd = a // 3, a % 3
                    if kd < 2:
                        dma_q[qi % 3].dma_start(
                            out=R9[a:a+1],
                            in_=AsbA[a:a+1, kd:kd+4, kh:kh+32, :])
                        qi += 1
                for a in range(9):
                    kh, kd = a // 3, a % 3
                    if kd == 2:
                        dma_q[qi % 3].dma_start(
                            out=R9[a:a+1, 0:3],
                            in_=AsbA[a:a+1, 2:5, kh:kh+32, :])
                        qi += 1
                        dma_q[qi % 3].dma_start(
                            out=R9[a:a+1, 3:4],
                            in_=AsbB[a:a+1, 0:1, kh:kh+32, :])
                        qi += 1
                ysb = ap.tile([1, 4096], f32)
                R9f = R9.rearrange("p a b c -> p (a b c)")
                for c in range(8):
                    Cps = ps6.tile([1, 512], f32, tag="cps")
                    nc.tensor.matmul(Cps[:], o9[:], R9f[:, c*512:(c+1)*512],
                                     start=True, stop=True, skip_group_check=True)
                    if c % 2 == 0:
                        nc.scalar.activation(ysb[:, c*512:(c+1)*512], Cps[:], IDENT)
                    else:
                        nc.vector.tensor_copy(ysb[:, c*512:(c+1)*512], Cps[:])
                    if c % 2 == 1:
                        # ship each completed 1KB pair immediately
                        dma_q[(c // 2) % 3].dma_start(
                            out=y_d[:, (c-1)*512:(c+1)*512],
                            in_=ysb[:, (c-1)*512:(c+1)*512])
    nc.leave_named_scope("L6", sc_l6[0], False)
    return nc

def _get_nc():
    if 'nc' not in _CACHE:
        _CACHE['nc'] = _build()
    return _CACHE['nc']

def run(inputs, trace=False):
    from concourse.bass_utils import run_bass_kernel_spmd
    P = _prep(inputs)
    in_maps = []
    for c in range(N_CORES):
        in_maps.append({
            "xx": P['xx'], "w1r": P['w1r'][c], "smat": P['smat'],
            "w2p": P['w2p'][c], "w2s": P['w2s'][c], "w3r": P['w3r'],
            "w4r": P['w4r'], "w5p": P['w5p'], "w5s": P['w5s'], "w6a": P['w6a'],
            "ones9": np.ones((9, 1), np.float16), "maskt": P['maskt'][c],
        })
    res = run_bass_kernel_spmd(_get_nc(), in_maps, list(range(N_CORES)), trace=trace)
    out = np.concatenate([res.results[c]["y"].reshape(4, 32, 32) for c in range(N_CORES)], axis=0)
    return out[None, None].astype(np.float32), res

def kernel(**inputs):
    out, _ = run(inputs, trace=False)
    return out

